# revision 1
# baseline (speedup 1.0000x reference)
"""Trainium2 Bass kernel for CustomMultiHeadAttention with relative position
bias (Music-Transformer skew), causal mask.

Sharding: pure data-parallel over batch — B=8 batches, one per NeuronCore.
Each core computes the full attention for its batch element; weights and Er
are replicated. No collectives.

Per-core pipeline:
  - PE-transpose weights (WqT/WkT/WvT/WoT) and activations (QT/KT/VT) so all
    projection matmuls contract d_in on partitions (fp32r, full PE rate).
  - qT/kT [768,1024] and v [1024,768](fp16) projections.
  - Per (s-strip, head): QEr strip via matmul against ErT; round-trip through
    a DRAM scratch with a diagonal access pattern to realize the skew
    (Srel[s,t] = QEr[s, S-1-s+t]); QK strip; add; causal mask on the diagonal
    tile via affine_select; Exp (scale=1/8) with accumulated row-sums;
    DMA-transpose of A tiles (fp16); A^T@V accumulated in PSUM; normalize by
    1/Z during eviction.
  - Per s-strip: PE-transpose attn strip, output projection, +bias, DMA out.
"""

import numpy as np

import concourse.bass as bass
import concourse.tile as tile
from concourse import bacc, mybir
from concourse.bass import AP
from concourse.bass_utils import run_bass_kernel_spmd
from concourse.masks import make_identity

N_CORES = 8
B, S, D, H, DK = 8, 1024, 768, 12, 64
NT = S // 128          # 8 s-tiles
NI = D // 128          # 6 d-blocks
f32 = mybir.dt.float32
f16 = mybir.dt.float16
f32r = mybir.dt.float32r
SCRATCH_RS = 1152      # row stride of DRAM skew scratch (>= 1024+128)
NEG_FILL = -1e30


def _transpose_into(nc, pools, src_dram, nrow, ncol, dst_tiles, ident, dst_col0=0):
    """Load [nrow, ncol] DRAM tensor natural, PE-transpose 128x128 tiles,
    evict into dst_tiles: list of [128, nrow] f32r tiles, one per 128-col
    block; dst column range [dst_col0, dst_col0+nrow)."""
    stg, ps = pools
    for rt in range(nrow // 128):
        nat = stg.tile([128, ncol], f32, tag="nat_stage")
        nc.sync.dma_start(out=nat[:], in_=src_dram.ap()[rt * 128:(rt + 1) * 128, :])
        for ct in range(ncol // 128):
            p = ps.tile([128, 512], f32, tag="ps_misc")
            nc.tensor.transpose(p[:, :128], nat[:, ct * 128:(ct + 1) * 128], ident[:])
            dst = dst_tiles[ct][:, dst_col0 + rt * 128: dst_col0 + (rt + 1) * 128]
            if (rt + ct) % 2 == 0:
                nc.scalar.copy(dst, p[:, :128])
            else:
                nc.vector.tensor_copy(dst, p[:, :128])


def build_nc():
    nc = bacc.Bacc("TRN2", target_bir_lowering=False, debug=False,
                   num_devices=N_CORES)

    Qb = nc.dram_tensor("Qb", [S, D], f32, kind="ExternalInput")
    Kb = nc.dram_tensor("Kb", [S, D], f32, kind="ExternalInput")
    Vb = nc.dram_tensor("Vb", [S, D], f32, kind="ExternalInput")
    Wq = nc.dram_tensor("Wq", [D, D], f32, kind="ExternalInput")
    Wk = nc.dram_tensor("Wk", [D, D], f32, kind="ExternalInput")
    Wv = nc.dram_tensor("Wv", [D, D], f32, kind="ExternalInput")
    Wo = nc.dram_tensor("Wo", [D, D], f32, kind="ExternalInput")
    bq = nc.dram_tensor("bq", [D], f32, kind="ExternalInput")
    bk = nc.dram_tensor("bk", [D], f32, kind="ExternalInput")
    bv = nc.dram_tensor("bv", [D], f32, kind="ExternalInput")
    bo = nc.dram_tensor("bo", [D], f32, kind="ExternalInput")
    Er = nc.dram_tensor("Er", [S, DK], f32, kind="ExternalInput")
    out = nc.dram_tensor("out", [S, D], f32, kind="ExternalOutput")

    # DRAM skew scratch: per-parity region [128, 4608] fp16 (packed strips)
    qer_dram = nc.dram_tensor("qer_scratch", [2 * 128 * 4608], f16)

    tensors = dict(Qb=Qb, Kb=Kb, Vb=Vb, Wq=Wq, Wk=Wk, Wv=Wv, Wo=Wo,
                   bq=bq, bk=bk, bv=bv, bo=bo, Er=Er, out=out,
                   qer_dram=qer_dram)
    with tile.TileContext(nc) as tc:
        _build_body(nc, tc, tensors)
    nc.compile()
    return nc


def _build_body(nc, tc, t):
    Qb, Kb, Vb = t["Qb"], t["Kb"], t["Vb"]
    Wq, Wk, Wv, Wo = t["Wq"], t["Wk"], t["Wv"], t["Wo"]
    bq, bk, bv, bo = t["bq"], t["bk"], t["bv"], t["bo"]
    Er, out, qer_dram = t["Er"], t["out"], t["qer_dram"]

    from contextlib import ExitStack
    with ExitStack() as ctx:
        persist = ctx.enter_context(tc.tile_pool(name="persist", bufs=1))
        ps_misc_cm = tc.tile_pool(name="ps_misc", bufs=6, space="PSUM")
        ps_misc = ps_misc_cm.__enter__()

        ident = persist.tile([128, 128], f32, tag="ident")
        make_identity(nc, ident[:])

        # ---- ErT duplicated into both partition halves [128, 1024] f32r ----
        erT2 = persist.tile([128, S], f32r, tag="erT2")
        with tc.tile_pool(name="er_stage", bufs=2) as erp:
            for et in range(NT):
                nat = erp.tile([128, DK], f32, tag="er_nat")
                nc.sync.dma_start(out=nat[:], in_=Er.ap()[et * 128:(et + 1) * 128, :])
                p = ps_misc.tile([128, 512], f32, tag="ps_misc")
                nc.tensor.transpose(p[:DK, :128], nat[:], ident[:])
                nc.scalar.copy(erT2[0:DK, et * 128:(et + 1) * 128], p[:DK, :128])
                nc.scalar.copy(erT2[DK:128, et * 128:(et + 1) * 128], p[:DK, :128])

        # ---- biases ----
        # bq/bk as per-partition columns per j-tile: [128, NI] each
        bq_col = persist.tile([128, NI], f32, tag="bq_col")
        bk_col = persist.tile([128, NI], f32, tag="bk_col")
        for jt in range(NI):
            nc.sync.dma_start(out=bq_col[:, jt:jt + 1],
                              in_=bq.ap()[jt * 128:(jt + 1) * 128].unsqueeze(1))
            nc.sync.dma_start(out=bk_col[:, jt:jt + 1],
                              in_=bk.ap()[jt * 128:(jt + 1) * 128].unsqueeze(1))
        # bv/bo broadcast to all 128 partitions
        bv_row = persist.tile([128, D], f32, tag="bv_row")
        nc.sync.dma_start(out=bv_row[:],
                          in_=AP(tensor=bv, offset=0, ap=[[0, 128], [1, D]]))
        bo_row = persist.tile([128, D], f32, tag="bo_row")
        nc.sync.dma_start(out=bo_row[:],
                          in_=AP(tensor=bo, offset=0, ap=[[0, 128], [1, D]]))

        # ---- persistent projection outputs ----
        qT = [persist.tile([128, S], f32r, tag=f"qT{i}", name=f"qT{i}") for i in range(NI)]
        kT = [persist.tile([128, S], f32r, tag=f"kT{i}", name=f"kT{i}") for i in range(NI)]
        v16 = [persist.tile([128, H * 65], f16, tag=f"v16{i}", name=f"v16{i}") for i in range(NT)]
        woT = [persist.tile([128, D], f32r, tag=f"woT{i}", name=f"woT{i}") for i in range(NI)]
        attn_outT = [persist.tile([128, S], f32r, tag=f"attn_outT{i}", name=f"attn_outT{i}")
                     for i in range(NI)]

        # ---- WoT (kept for the whole kernel) ----
        with tc.tile_pool(name="stage_wo", bufs=2) as stg:
            _transpose_into(nc, (stg, ps_misc), Wo, D, D, woT, ident)

        # ---- qT / kT projections ----
        for X, W, bias_col, xT_out in ((Qb, Wq, bq_col, qT), (Kb, Wk, bk_col, kT)):
            with tc.tile_pool(name="stage_x", bufs=2) as stg, \
                 tc.tile_pool(name="stage_big", bufs=1) as big:
                wT = [big.tile([128, D], f32r, tag=f"wT{i}", name=f"wT{i}") for i in range(NI)]
                xT = [big.tile([128, S], f32r, tag=f"xT{i}", name=f"xT{i}") for i in range(NI)]
                _transpose_into(nc, (stg, ps_misc), W, D, D, wT, ident)
                _transpose_into(nc, (stg, ps_misc), X, S, D, xT, ident)
                for jt in range(NI):
                    for sh in range(2):
                        p = ps_misc.tile([128, 512], f32, tag="ps_misc")
                        for ib in range(NI):
                            nc.tensor.matmul(
                                p[:],
                                wT[ib][:, jt * 128:(jt + 1) * 128],
                                xT[ib][:, sh * 512:(sh + 1) * 512],
                                start=(ib == 0), stop=(ib == NI - 1),
                            )
                        nc.vector.tensor_scalar_add(
                            xT_out[jt][:, sh * 512:(sh + 1) * 512], p[:],
                            bias_col[:, jt:jt + 1],
                        )

        # ---- v projection (natural layout, fp16, 65-col interleave, +bv) ----
        for tt in range(NT):
            nc.vector.memset(v16[tt][:], 1.0)
        with tc.tile_pool(name="stage_x", bufs=2) as stg, \
             tc.tile_pool(name="stage_big", bufs=1) as big:
            wT = [big.tile([128, D], f32r, tag=f"wT{i}", name=f"wT{i}") for i in range(NI)]
            vT = [big.tile([128, S], f32r, tag=f"xT{i}", name=f"vT{i}") for i in range(NI)]
            _transpose_into(nc, (stg, ps_misc), Wv, D, D, wT, ident)
            _transpose_into(nc, (stg, ps_misc), Vb, S, D, vT, ident)
            for tt in range(NT):
                for js, w in ((0, 512), (512, 256)):
                    p = ps_misc.tile([128, 512], f32, tag="ps_misc")
                    for ib in range(NI):
                        nc.tensor.matmul(
                            p[:, :w],
                            vT[ib][:, tt * 128:(tt + 1) * 128],
                            wT[ib][:, js:js + w],
                            start=(ib == 0), stop=(ib == NI - 1),
                        )
                    hh0 = js // 64
                    nh = w // 64
                    dst3 = v16[tt][:, :].rearrange(
                        "p (a b) -> p a b", b=65)[:, hh0:hh0 + nh, 0:64]
                    nc.vector.tensor_add(
                        dst3, p[:, :w].rearrange("p (a b) -> p a b", b=64),
                        bv_row[:, js:js + w].rearrange("p (a b) -> p a b", b=64),
                    )

        ps_misc_cm.__exit__(None, None, None)

        # ---- attention (whole-head, transposed scores) ----
        # srel_nat packed-causal offsets: strip si at column off_si
        OFFS = [0]
        for si in range(NT):
            OFFS.append(OFFS[-1] + 128 * (si + 1))
        SREL_W = OFFS[-1]          # 4608
        # A16T packed-causal: block ti (width 1024-128*ti) at col OT[ti]
        OT = [0]
        for ti in range(NT):
            OT.append(OT[-1] + S - 128 * ti)
        AW = OT[-1]                # 4608
        with tc.tile_pool(name="work", bufs=2) as work, \
             tc.tile_pool(name="ps_a", bufs=2, space="PSUM") as ps_a, \
             tc.tile_pool(name="ps_qk", bufs=2, space="PSUM") as ps_qk, \
             tc.tile_pool(name="ps_av", bufs=1, space="PSUM") as ps_av:
            ones1f = work.tile([1, 64], f32, tag="ones1f", bufs=1)
            nc.vector.memset(ones1f[:], 1.0)
            ones1 = work.tile([1, 64], f32r, tag="ones1", bufs=1)
            nc.vector.tensor_copy(ones1[:], ones1f[:])
            ident32r = work.tile([128, 128], f32r, tag="ident32r", bufs=1)
            nc.vector.tensor_copy(ident32r[:], ident[:])
            for hp in range(H // 2):
                jb = hp
                # per-head contexts: (jr, reg, tiles...)
                ctxs = []
                for idx in range(2):
                    h = 2 * hp + idx
                    jr = 64 * idx
                    reg = idx * 128 * SREL_W
                    ctxs.append(dict(h=h, jr=jr, reg=reg))

                # --- QEr strips, interleaved across the pair ---
                for c in ctxs:
                    c["srel_out"] = work.tile([128, SREL_W], f16, tag="srel_out",
                                              name=f"srel_out{c['h']}")
                for si in range(NT):
                    Wcw = 128 * (si + 1)
                    e0 = S - Wcw
                    for cs in range(0, Wcw, 512):
                        w = min(512, Wcw - cs)
                        for c in ctxs:
                            jr = c["jr"]
                            qTs = qT[jb][jr:jr + 64, si * 128:(si + 1) * 128]
                            p = ps_a.tile([128, 512], f32, tag="psa")
                            nc.tensor.matmul(p[:, :w], qTs,
                                             erT2[jr:jr + 64, e0 + cs:e0 + cs + w],
                                             start=True, stop=True)
                            dstp = c["srel_out"][:, OFFS[si] + cs:OFFS[si] + cs + w]
                            if (si + c["jr"] // 64) % 2 == 0:
                                nc.vector.tensor_copy(dstp, p[:, :w])
                            else:
                                nc.scalar.copy(dstp, p[:, :w])
                for c in ctxs:
                    dst = AP(tensor=qer_dram, offset=c["reg"],
                             ap=[[SREL_W, 128], [1, SREL_W]])
                    nc.sync.dma_start(out=dst, in_=c["srel_out"][:])

                # --- skew reads with f16 -> f32r cast on SWDGE ---
                for c in ctxs:
                    c["srel32"] = work.tile([128, SREL_W], f32r, tag="srel32",
                                            name=f"srel32_{c['h']}", bufs=2)
                for si in range(NT):
                    Wcw = 128 * (si + 1)
                    for c in ctxs:
                        skew = AP(tensor=qer_dram,
                                  offset=c["reg"] + OFFS[si] + 127,
                                  ap=[[SREL_W - 1, 128], [1, Wcw]])
                        nc.gpsimd.dma_start(
                            out=c["srel32"][:, OFFS[si]:OFFS[si] + Wcw], in_=skew)

                # --- QK + srel-transpose-accumulate + exp ---
                for c in ctxs:
                    c["A16T"] = work.tile([128, AW], f16, tag="A16T",
                                          name=f"A16T{c['h']}")
                for ti in range(NT):
                    s0 = 128 * ti
                    w = S - s0
                    for cs in range(0, w, 512):
                        cw = min(512, w - cs)
                        pqks = []
                        for c in ctxs:
                            jr = c["jr"]
                            pqk = ps_qk.tile([128, 512], f32, tag="qk", bufs=2,
                                             name=f"pqk{c['h']}")
                            nc.tensor.matmul(pqk[:, :cw],
                                             kT[jb][jr:jr + 64, ti * 128:(ti + 1) * 128],
                                             qT[jb][jr:jr + 64, s0 + cs:s0 + cs + cw],
                                             start=True, stop=False)
                            pqks.append(pqk)
                        for c, pqk in zip(ctxs, pqks):
                            nch = cw // 128
                            for k in range(nch):
                                sic = ti + (cs + k * 128) // 128
                                nc.tensor.matmul(
                                    pqk[:, k * 128:(k + 1) * 128].bitcast(f32r),
                                    c["srel32"][:, OFFS[sic] + 128 * ti:
                                                OFFS[sic] + 128 * ti + 128],
                                    ident32r[:],
                                    is_transpose=True,
                                    start=False, stop=(k == nch - 1))
                            nc.scalar.activation(
                                c["A16T"][:, OT[ti] + cs:OT[ti] + cs + cw],
                                pqk[:, :cw],
                                mybir.ActivationFunctionType.Exp, scale=0.125)
                    for c in ctxs:
                        nc.gpsimd.affine_select(
                            out=c["A16T"][:, OT[ti]:OT[ti] + 128],
                            in_=c["A16T"][:, OT[ti]:OT[ti] + 128],
                            pattern=[[1, 128]], compare_op=mybir.AluOpType.is_ge,
                            fill=0.0, base=0, channel_multiplier=-1,
                        )

                # --- AV + normalize per head ---
                for c in ctxs:
                    h, jr = c["h"], c["jr"]
                    pav = ps_av.tile([65, S], f32, tag="av", name=f"pav{h}", bufs=2)
                    for ti in range(NT):
                        v16s = v16[ti][:, h * 65:(h + 1) * 65]
                        pieces = ([(128 * ti, 512 - 128 * ti), (512, 512)]
                                  if ti < 4 else [(128 * ti, 1024 - 128 * ti)])
                        for (s0, w) in pieces:
                            stop = (ti == 3) if s0 < 512 else (ti == 7)
                            nc.tensor.matmul(pav[:, s0:s0 + w], v16s,
                                             c["A16T"][:, OT[ti] + s0 - 128 * ti:
                                                       OT[ti] + s0 - 128 * ti + w],
                                             start=(ti == 0), stop=stop)
                    rZ_row = work.tile([1, S], f32r, tag="rZ_row", name=f"rZ{h}")
                    with nc.allow_low_precision(reason="f32r == f32 bits"):
                        nc.vector.reciprocal(rZ_row[:], pav[64:65, :])
                    rzb = work.tile([64, S], f32, tag="rzb_sb", bufs=1,
                                    name=f"rzb{h}")
                    for i in range(2):
                        przi = ps_qk.tile([64, 512], f32, tag="qk",
                                          name=f"prz{h}_{i}", bufs=2)
                        nc.tensor.matmul(przi[:], ones1[:],
                                         rZ_row[:, i * 512:(i + 1) * 512],
                                         start=True, stop=True)
                        nc.vector.tensor_copy(rzb[:, i * 512:(i + 1) * 512], przi[:])
                    if h % 2 == 0:
                        nc.vector.tensor_mul(attn_outT[jb][0:64, :], pav[0:64, :],
                                             rzb[:])
                    else:
                        odd_tmp = work.tile([64, S], f32r, tag="odd_tmp",
                                            name=f"ot{h}")
                        nc.vector.tensor_mul(odd_tmp[:], pav[0:64, :], rzb[:])
                        nc.sync.dma_start(out=attn_outT[jb][64:128, :],
                                          in_=odd_tmp[:])

        # ---- output projection + final transpose ----
        with tc.tile_pool(name="ps_o", bufs=2, space="PSUM") as ps_o, \
             tc.tile_pool(name="stage_o", bufs=2) as stg:
            for sh in range(2):
                oT = [stg.tile([128, 512], f32, tag=f"oT{jt}", name=f"oT{jt}")
                      for jt in range(NI)]
                for jt in range(NI):
                    p = ps_o.tile([128, 512], f32, tag="o")
                    for ib in range(NI):
                        nc.tensor.matmul(p[:], woT[ib][:, jt * 128:(jt + 1) * 128],
                                         attn_outT[ib][:, sh * 512:(sh + 1) * 512],
                                         start=(ib == 0), stop=(ib == NI - 1))
                    nc.scalar.copy(oT[jt][:], p[:])
                for st in range(4):
                    outs = stg.tile([128, D], f32, tag="out_strip")
                    for jt in range(NI):
                        pt = ps_o.tile([128, 512], f32, tag="o")
                        nc.tensor.transpose(pt[:, :128], oT[jt][:, st * 128:(st + 1) * 128],
                                            ident[:])
                        nc.vector.tensor_add(outs[:, jt * 128:(jt + 1) * 128],
                                             pt[:, :128], bo_row[:, jt * 128:(jt + 1) * 128])
                    row = (sh * 4 + st) * 128
                    nc.sync.dma_start(out=out.ap()[row:row + 128, :], in_=outs[:])


_NC = None


def kernel(**inputs):
    global _NC
    if _NC is None:
        _NC = build_nc()
    Q = np.ascontiguousarray(np.asarray(inputs["Q"], dtype=np.float32))
    K = np.ascontiguousarray(np.asarray(inputs["K"], dtype=np.float32))
    V = np.ascontiguousarray(np.asarray(inputs["V"], dtype=np.float32))
    shared = {
        name: np.ascontiguousarray(np.asarray(inputs[name], dtype=np.float32))
        for name in ("Wq", "Wk", "Wv", "Wo", "bq", "bk", "bv", "bo", "Er")
    }
    in_maps = [
        {"Qb": Q[c], "Kb": K[c], "Vb": V[c], **shared} for c in range(N_CORES)
    ]
    global _last_in_maps
    _last_in_maps = in_maps
    res = run_bass_kernel_spmd(_NC, in_maps, list(range(N_CORES)))
    return np.stack([res.results[c]["out"] for c in range(N_CORES)], axis=0)

